# revision 7
# baseline (speedup 1.0000x reference)
import os
import sys
import time

sys.path.insert(0, '/opt/trn_rl_repo')
import numpy as np
import concourse.bass as bass
import concourse.mybir as mybir
import concourse.tile as tile
from concourse import bacc
from concourse.bass_utils import run_bass_kernel_spmd
from concourse.masks import make_identity

F32, F16 = mybir.dt.float32, mybir.dt.float16
AOP = mybir.AluOpType
AF = mybir.ActivationFunctionType

L, H, DIN, Bfull = 2, 512, 512, 64
B = 16          # batch per core (64 over 4 quarters; 2 directions -> 8 cores)
KC, NS, SW = 4, 4, 384
NCORES = 8

# gate permutation: g~ = 384j + 128u + c  <->  original gate row 512u + 128j + c
_jj, _uu, _cc = np.meshgrid(np.arange(NS), np.arange(3), np.arange(128), indexing='ij')
PERM = np.empty(3 * H, np.int64)
PERM[(SW * _jj + 128 * _uu + _cc).ravel()] = (H * _uu + 128 * _jj + _cc).ravel()
N_COLS = np.array([SW * j + 256 + c for j in range(NS) for c in range(128)])

_idb17 = np.zeros((17, 32), np.float16)
_idb17[:16, :16] = np.eye(16)
_idb17[16, :] = 1.0

LAST_TIME_NS = None
LAST_RES = None


def _build_program(T):
    nc = bacc.Bacc("TRN2", target_bir_lowering=False, debug=False)
    NT = T // 8

    d_xT = nc.dram_tensor("xT", [T, 128, KC, B], F16, kind="ExternalInput").ap()
    d_h0T = nc.dram_tensor("h0T", [L, KC, 128, B], F16, kind="ExternalInput").ap()
    d_h0pf = nc.dram_tensor("h0pf", [L, 128, 128], F32, kind="ExternalInput").ap()
    d_wihT = nc.dram_tensor("wihT", [L, KC, 128, 3 * H], F16, kind="ExternalInput").ap()
    d_whhT = nc.dram_tensor("whhT", [L, KC, 128, 3 * H], F16, kind="ExternalInput").ap()
    d_brow = nc.dram_tensor("brow", [L, 1, 3 * H], F16, kind="ExternalInput").ap()
    d_bhhn = nc.dram_tensor("bhhn", [L, 1, 512], F16, kind="ExternalInput").ap()
    d_idb = nc.dram_tensor("idb17", [17, 32], F16, kind="ExternalInput").ap()
    d_ones = nc.dram_tensor("ones32", [1, 32], F16, kind="ExternalInput").ap()
    d_yout = nc.dram_tensor("yout", [T, 128, 128], F32, kind="ExternalOutput").ap()
    DEBUG = bool(os.environ.get("GRU_DEBUG"))
    if DEBUG:
        d_gxdbg = nc.dram_tensor("gxdbg", [3, T, B, 512], F16, kind="ExternalOutput").ap()
        d_y0dbg = nc.dram_tensor("y0dbg", [T, 128, KC, B], F16, kind="ExternalOutput").ap()
    d_hN = nc.dram_tensor("hN", [L, 128, 128], F32, kind="ExternalOutput").ap()

    with tile.TileContext(nc) as tc:
        with (
            tc.tile_pool(name="singles", bufs=1) as singles,
            tc.tile_pool(name="work", bufs=3) as work,
            tc.tile_pool(name="proj", bufs=2) as proj,
            tc.tile_pool(name="ps", bufs=2, space="PSUM") as pp,
            tc.tile_pool(name="psT", bufs=2, space="PSUM") as ppT,
            tc.tile_pool(name="psP", bufs=2, space="PSUM") as ppP,
            tc.tile_pool(name="dram", bufs=1, space="DRAM") as dpool,
        ):
            wih_sb = singles.tile([128, L, KC, 3 * H], F16)
            nc.sync.dma_start(out=wih_sb, in_=d_wihT.rearrange("l kc hp g -> hp l kc g"))
            whh_sb = singles.tile([128, L, KC, 3 * H], F16)
            nc.sync.dma_start(out=whh_sb, in_=d_whhT.rearrange("l kc hp g -> hp l kc g"))
            bhhn_sb = singles.tile([1, L, 512], F16)
            nc.sync.dma_start(out=bhhn_sb, in_=d_bhhn.rearrange("l o n -> o l n"))
            idb = singles.tile([17, 32], F16)
            nc.sync.dma_start(out=idb, in_=d_idb)
            ones32 = singles.tile([1, 32], F16)
            nc.sync.dma_start(out=ones32, in_=d_ones)
            id128 = singles.tile([128, 128], F32)
            make_identity(nc, id128)

            gxb = [[singles.tile([17, NS, SW], F16, name=f"gxb{l}_{i}") for i in range(2)]
                   for l in range(L)]
            hT = [[singles.tile([128, KC, B], F16, name=f"hT{l}_{i}") for i in range(2)]
                  for l in range(L)]
            hpf = [[singles.tile([128, 128], F32, name=f"hpf{l}_{i}") for i in range(2)]
                   for l in range(L)]
            for l in range(L):
                for i in range(2):
                    nc.sync.dma_start(out=gxb[l][i][16:17, :, :],
                                      in_=d_brow[l].rearrange("o (j w) -> o j w", j=NS))
                nc.sync.dma_start(out=hT[l][0], in_=d_h0T[l].rearrange("kc hp b -> hp kc b"))
                nc.sync.dma_start(out=hpf[l][0], in_=d_h0pf[l])

            gxd = [dpool.tile([3, T, B, 512], F16, name=f"gxd{l}") for l in range(L)]
            y0T = dpool.tile([T, 128, KC, B], F16, name="y0T")

            def proj_rowtile(l, src, t0):
                # gx[l] rows [t0, t0+8) = src[t0:t0+8] @ WihT[l];  src: [T, 128, KC, B] fp16
                x_sb = proj.tile([128, KC, 8, B], F16, tag="x_sb")
                nc.sync.dma_start(out=x_sb,
                                  in_=src[bass.ds(t0, 8)].rearrange("t hp kc b -> hp kc t b"))
                for n3 in range(3):
                    ps = ppP.tile([128, 512], F32, tag="psP")
                    for kc in range(KC):
                        nc.tensor.matmul(ps, lhsT=x_sb[:, kc],
                                         rhs=wih_sb[:, l, kc, 512 * n3:512 * n3 + 512],
                                         start=(kc == 0), stop=(kc == KC - 1))
                    cp = proj.tile([128, 512], F16, tag="cp")
                    nc.scalar.copy(out=cp, in_=ps)
                    nc.sync.dma_start(
                        out=gxd[l][n3, bass.ds(t0, 8)].rearrange("t b n -> (t b) n"),
                        in_=cp)

            def rec_step(l, t, parity):
                # one step of layer l; t may be int or RuntimeValue
                cur, nxt = parity, 1 - parity
                gb = gxb[l][parity]
                g_in = gxd[l][:, bass.ds(t, 1)].rearrange("n t b w -> (t b) n w")
                g_out = gb[0:B, :, :].rearrange("b j w -> b (j w)").rearrange(
                    "b (n m) -> b n m", n=3)
                nc.sync.dma_start(out=g_out, in_=g_in)
                gh = pp.tile([128, 512], F32, tag=f"gh{l}")
                for j in range(NS):
                    tp = (0, 32 * j)
                    sl = gh[32 * j:32 * j + 32, :]
                    nc.tensor.matmul(sl[:, 0:256], lhsT=idb, rhs=gb[:, j, 0:256],
                                     start=True, stop=False, tile_position=tp)
                    nc.tensor.matmul(sl[:, 384:512], lhsT=idb, rhs=gb[:, j, 256:384],
                                     start=False, stop=False, tile_position=tp,
                                     skip_group_check=True)
                    nc.tensor.matmul(sl[:, 256:384], lhsT=ones32,
                                     rhs=bhhn_sb[:, l, 128 * j:128 * j + 128],
                                     start=False, stop=False, tile_position=tp,
                                     skip_group_check=True)
                for kc in range(KC):
                    for j in range(NS):
                        nc.tensor.matmul(gh[32 * j:32 * j + B, 0:384], lhsT=hT[l][cur][:, kc, :],
                                         rhs=whh_sb[:, l, kc, SW * j:SW * j + SW],
                                         start=False, stop=(kc == KC - 1),
                                         tile_position=(0, 32 * j), skip_group_check=True)
                rz = work.tile([128, 256], F32, tag=f"rz{l}")
                nc.scalar.activation(rz, gh[:, 0:256], AF.Sigmoid)
                tt = work.tile([128, 128], F32, tag=f"tt{l}")
                nc.vector.tensor_tensor(tt, rz[:, 0:128], gh[:, 256:384], op=AOP.mult)
                t2 = work.tile([128, 128], F32, tag=f"t2{l}")
                nc.vector.tensor_tensor(t2, tt, gh[:, 384:512], op=AOP.add)
                nsb = work.tile([128, 128], F32, tag=f"nsb{l}")
                nc.scalar.activation(nsb, t2, AF.Tanh)
                zh = work.tile([128, 128], F32, tag=f"zh{l}")
                nc.gpsimd.tensor_tensor(zh, rz[:, 128:256], hpf[l][cur], op=AOP.mult)
                w = work.tile([128, 128], F32, tag=f"w{l}")
                nc.gpsimd.tensor_scalar(w, rz[:, 128:256], -1.0, 1.0, op0=AOP.mult, op1=AOP.add)
                u = work.tile([128, 128], F32, tag=f"u{l}")
                nc.vector.tensor_tensor(u, nsb, w, op=AOP.mult)
                nc.vector.tensor_tensor(hpf[l][nxt], u, zh, op=AOP.add)
                psT = ppT.tile([128, 128], F32, tag="psT")
                nc.tensor.transpose(psT, hpf[l][nxt], id128)
                nc.scalar.copy(out=hT[l][nxt],
                               in_=psT.rearrange("p (kc q) -> p kc q", kc=KC)[:, :, 0:B])
                if l == 0:
                    nc.sync.dma_start(
                        out=y0T[bass.ds(t, 1)].rearrange("t hp kc b -> (t hp) kc b"),
                        in_=hT[l][nxt])
                else:
                    nc.sync.dma_start(
                        out=d_yout[bass.ds(t, 1)].rearrange("t p c -> (t p) c"),
                        in_=hpf[l][nxt])

            def emit_body(iv, do_l0, do_p1, do_l1):
                # iv: t-base (multiple of 8), int or RuntimeValue
                if do_l0:
                    for ui in range(8):
                        rec_step(0, iv + ui, ui % 2)
                if do_p1:
                    proj_rowtile(1, y0T, iv - 8)
                if do_l1:
                    for ui in range(8):
                        rec_step(1, iv - 16 + ui, ui % 2)

            # phase P0: full layer-0 projection (static)
            for rt in range(NT):
                proj_rowtile(0, d_xT, rt * 8)

            tc.strict_bb_all_engine_barrier()
            emit_body(0, True, False, False)
            tc.strict_bb_all_engine_barrier()
            emit_body(8, True, True, False)
            tc.strict_bb_all_engine_barrier()
            with tc.For_i(16, T, 8) as iv:
                emit_body(iv, True, True, True)
            tc.strict_bb_all_engine_barrier()
            emit_body(T, False, True, True)
            tc.strict_bb_all_engine_barrier()
            emit_body(T + 8, False, False, True)

            for l in range(L):
                nc.sync.dma_start(out=d_hN[l], in_=hpf[l][0])
            if DEBUG:
                nc.sync.dma_start(out=d_gxdbg, in_=gxd[0])
                nc.sync.dma_start(out=d_y0dbg, in_=y0T)

    nc.compile()
    return nc


def kernel(x, encoder_h, Wih_f, Whh_f, bih_f, bhh_f, Wih_b, Whh_b, bih_b, bhh_b):
    global LAST_TIME_NS, LAST_RES
    x = np.asarray(x, np.float32)
    encoder_h = np.asarray(encoder_h, np.float32)
    T = x.shape[1]
    nc = _build_program(T)

    Wih = [np.asarray(Wih_f, np.float32), np.asarray(Wih_b, np.float32)]
    Whh = [np.asarray(Whh_f, np.float32), np.asarray(Whh_b, np.float32)]
    bih = [np.asarray(bih_f, np.float32), np.asarray(bih_b, np.float32)]
    bhh = [np.asarray(bhh_f, np.float32), np.asarray(bhh_b, np.float32)]

    in_maps = []
    for c in range(NCORES):
        d, q = c // 4, c % 4
        bs = slice(16 * q, 16 * q + 16)
        xd = x[bs] if d == 0 else x[bs, ::-1]
        xT = np.ascontiguousarray(
            xd.transpose(1, 2, 0).reshape(T, KC, 128, B).transpose(0, 2, 1, 3)
        ).astype(np.float16)
        wihT = np.empty((L, KC, 128, 3 * H), np.float16)
        whhT = np.empty((L, KC, 128, 3 * H), np.float16)
        brow = np.empty((L, 1, 3 * H), np.float16)
        bhhn = np.empty((L, 1, 512), np.float16)
        h0T = np.empty((L, KC, 128, B), np.float16)
        h0pf = np.zeros((L, 128, 128), np.float32)
        for l in range(L):
            wihT[l] = Wih[d][l][PERM].T.reshape(KC, 128, 3 * H)
            whhT[l] = Whh[d][l][PERM].T.reshape(KC, 128, 3 * H)
            br = (bih[d][l] + bhh[d][l])[PERM].copy()
            br[N_COLS] = bih[d][l][PERM][N_COLS]
            brow[l, 0] = br
            bhhn[l, 0] = bhh[d][l][PERM][N_COLS]
            h0 = encoder_h[l, bs, 512 * d:512 * d + 512]
            h0T[l] = h0.T.reshape(KC, 128, B)
            for j in range(NS):
                h0pf[l, 32 * j:32 * j + B, :] = h0[:, 128 * j:128 * j + 128]
        in_maps.append({
            "xT": xT, "h0T": h0T, "h0pf": h0pf, "wihT": wihT, "whhT": whhT,
            "brow": brow, "bhhn": bhhn, "idb17": _idb17,
            "ones32": np.ones((1, 32), np.float16),
        })

    t0 = time.perf_counter()
    res = run_bass_kernel_spmd(nc, in_maps, core_ids=list(range(NCORES)))
    LAST_TIME_NS = (time.perf_counter() - t0) * 1e9
    LAST_RES = res

    out = np.empty((Bfull, T, 2 * H), np.float32)
    hout = np.empty((L, Bfull, 2 * H), np.float32)
    for c in range(NCORES):
        d, q = c // 4, c % 4
        bs = slice(16 * q, 16 * q + 16)
        yo = res.results[c]["yout"]          # [T, 128, 128]
        ybl = yo.reshape(T, NS, 32, 128)[:, :, :B, :].transpose(2, 0, 1, 3).reshape(B, T, H)
        if d == 0:
            out[bs, :, 0:H] = ybl
        else:
            out[bs, :, H:2 * H] = ybl[:, ::-1, :]
        hN = res.results[c]["hN"]            # [L, 128, 128]
        hbl = hN.reshape(L, NS, 32, 128)[:, :, :B, :].transpose(0, 2, 1, 3).reshape(L, B, H)
        hout[:, bs, 512 * d:512 * d + 512] = hbl
    return out, hout
